# revision 44
# baseline (speedup 1.0000x reference)
"""Trainium2 Bass kernel for nn_Attention_73486890434886.

Gated 8-head attention (head_dim 32) with a full [8, 2048, 2048] attention
bias, batch 1, q_len = kv_len = 2048, fused QG / KV projections and a gated
output projection.

Strategy (8 NeuronCores, SPMD, no collectives):
  - Shard the 2048 q rows across the 8 cores (256 rows each).  Every core
    computes all 8 heads for its q-slice; kv-side projections are replicated
    (cheap), which removes the output all-reduce entirely.
  - All attention math is in a "transposed" orientation so no on-device
    transposes are needed: logits^T [kv, q] come from k-stationary x
    (zero-padded per-head) q-moving matmuls, the host-pretransposed bias is
    injected into PSUM with an identity-stationary matmul (or a DVE add),
    exp runs on the scalar engine, and attn@v consumes exp(logits^T) as the
    matmul moving operand producing attn_out^T [c, q].  Softmax denominators
    ride along as a ones-column appended to the v stationary (M=33).
  - bf16 on the TensorEngine (fp32 PSUM accumulation), f32 softmax on ACT.
"""

import numpy as np
import ml_dtypes

import concourse.bass as bass
import concourse.mybir as mybir
import concourse.tile as tile
from concourse import bacc
from concourse.bass_utils import run_bass_kernel_spmd

BF16 = ml_dtypes.bfloat16
F8 = ml_dtypes.float8_e4m3

# Problem shapes (hardcoded per the task statement).
B, QL, KVL, D, H, C, O = 1, 2048, 2048, 256, 8, 32, 256
NCORES = 8
QS = QL // NCORES          # 256 q rows per core
NKC = KVL // 128           # 16 kv chunks of 128
NG = 2                     # head groups (0-3, 4-7)
HPG = H // NG              # heads per group = 4

# Head-pair banks: group g, bank b -> heads (4g+2b, 4g+2b+1); within an acc
# bank the two heads sit at partitions 0..32 and 64..96 (numer rows +
# trailing rowsum row from the ones column of the v stationary).

f32 = mybir.dt.float32
bf16 = mybir.dt.bfloat16
i16 = mybir.dt.int16
f8e4 = mybir.dt.float8e4

# Schraudolph bf16-exp constants: exp(x) ~= bitcast_bf16(int16(A16*x + B16)).
# A16 = 2^7 * log2(e); B16 = 127*2^7 + delta, delta tuned to center the
# mantissa-linear approximation error (+-3%).  The bias tensor for DVE-path
# banks is premultiplied on the host: bTs = int16(A16*bias + B16), so one
# scalar_tensor_tensor op computes exp(logits + bias) for the whole tile.
A16 = 184.6650390625
B16 = 16253.28


# Chunks whose exp runs on DVE (Schraudolph, fused int16 bias) instead of
# PE-inject + ACT exp.  Every 4th chunk (both banks) keeps DVE (which also
# carries the projection drains and gating) level with ACT, and lets the
# remaining chunks carry fp8 bias (half the DMA footprint of bf16).
def _dve_exp(g, c, b2):
    return c % 4 == 3


# fp8-bias chunk index: chunks with c % 4 != 3, renumbered densely.
def _b8_idx(c):
    return c - c // 4


# ---------------------------------------------------------------------------
# Host-side packing: everything is laid out partition-major so every DMA is a
# straight contiguous copy.
# ---------------------------------------------------------------------------

def _pack_shared(inputs):
    kv = np.asarray(inputs["kv_inputs"], np.float32)[0]        # [KVL, D]
    qg_w = np.asarray(inputs["qg_weights"], np.float32)[:, 0]  # [D, H, 2C]
    qg_b = np.asarray(inputs["qg_bias"], np.float32)[0, :, 0]  # [H, 2C]
    kv_w = np.asarray(inputs["kv_weights"], np.float32)[:, 0]  # [D, H, 2C]
    kv_b = np.asarray(inputs["kv_bias"], np.float32)[0, :, 0]  # [H, 2C]
    o_w = np.asarray(inputs["o_weights"], np.float32)[0]       # [H, C, O]
    o_b = np.asarray(inputs["o_bias"], np.float32)[:, 0]       # [O]

    scale = C ** -0.5

    # Per-head zero-padded q weights: stationary tile h has w_q in column
    # block 32h'..32h'+32, zeros elsewhere, so the logits matmul can contract
    # over the full 128 partitions of the packed k tile without mixing heads.
    wq_full = qg_w[:, :, :C] * scale           # [D, H, C]
    wq_pad = np.zeros((D, H, 128), np.float32)
    for h in range(H):
        hp = h % HPG
        wq_pad[:, h, 32 * hp:32 * hp + 32] = wq_full[:, h, :]
    wq_pad = wq_pad.reshape(2, 128, H, 128).transpose(1, 2, 0, 3)  # [128,H,kc,128]

    # Gate weights in head-pair "bank" layout: tile (g,b) has head 4g+2b at
    # columns 0..32 and head 4g+2b+1 at columns 64..96, zeros elsewhere.
    wg_full = qg_w[:, :, C:]                   # [D, H, C]
    wg_pair = np.zeros((D, NG * 2, 128), np.float32)
    gbn = np.zeros((128, NG * 2), np.float32)  # gate_bias / 2, same layout
    for g in range(NG):
        for b in range(2):
            for j in range(2):
                h = 4 * g + 2 * b + j
                wg_pair[:, 2 * g + b, 64 * j:64 * j + C] = wg_full[:, h, :]
                gbn[64 * j:64 * j + C, 2 * g + b] = 0.5 * qg_b[h, C:]
    wg_pair = wg_pair.reshape(2, 128, NG * 2, 128).transpose(1, 2, 0, 3)

    # Packed k weights: [128, NG, kc, 128] with m = h'*C + c.
    wk = kv_w[:, :, :C].reshape(D, NG, HPG * C)
    wk = wk.transpose(1, 0, 2).reshape(NG, 2, 128, HPG * C).transpose(2, 0, 1, 3)

    wv = kv_w[:, :, C:].reshape(D, H * C)
    wv = wv.reshape(2, 128, H * C).transpose(1, 0, 2)          # [128, 2, 256]

    qb_full = qg_b[:, :C] * scale
    qbp = np.zeros((128, H), np.float32)
    for h in range(H):
        hp = h % HPG
        qbp[32 * hp:32 * hp + 32, h] = qb_full[h]
    kb = kv_b[:, :C].reshape(NG, 128).T                        # [128, 2]
    vbb = np.broadcast_to(kv_b[:, C:].reshape(1, H * C), (128, H * C)).copy()

    # o weights in bank layout with zero rows outside the two 32-row head
    # blocks (kills the junk rows of the gated-attention tile).
    ow = np.zeros((128, NG * 2, 2, 128), np.float32)
    o_flat = o_w.reshape(H * C, O)             # [(h,c), o]
    for g in range(NG):
        for b in range(2):
            for j in range(2):
                h = 4 * g + 2 * b + j
                for t in range(2):
                    ow[64 * j:64 * j + C, 2 * g + b, t, :] = \
                        o_flat[h * C:(h + 1) * C, t * 128:(t + 1) * 128]
    ob = o_b.reshape(2, 128).T                 # [128, 2]

    kviT = kv.T.reshape(2, 128, KVL).transpose(1, 0, 2)        # [128, 2, KVL]

    iden = np.eye(128, dtype=np.float32)
    ind2 = np.zeros((128, 128), np.float32)    # row broadcast: m <- 64*(m//64)+32
    for m in range(128):
        ind2[64 * (m // 64) + 32, m] = 1.0

    # Weight packs split by consumption time so the DMA stream delivers the
    # startup-critical tensors (k/v weights, kv inputs, q weights) before the
    # bulk bias and the end-of-kernel output weights.
    wka = np.concatenate([
        wk.reshape(128, -1), wv.reshape(128, -1), iden,
    ], axis=1)                                  # [128, 1152]
    wqg = np.concatenate([
        wq_pad.reshape(128, -1), wg_pair.reshape(128, -1),
    ], axis=1)                                  # [128, 3072]
    wob = np.concatenate([ow.reshape(128, -1), ind2], axis=1)  # [128, 1152]
    wpk32 = np.concatenate([qbp, gbn, kb, vbb, ob], axis=1)  # [128, 272]
    return {
        "kviT": kviT.astype(BF16),
        "wka": np.ascontiguousarray(wka).astype(BF16),
        "wqg": np.ascontiguousarray(wqg).astype(BF16),
        "wob": np.ascontiguousarray(wob).astype(BF16),
        "wpk32": np.ascontiguousarray(wpk32).astype(np.float32),
        "i8": iden.astype(F8),
    }


def _pack_core(inputs, core):
    qs = core * QS
    q = np.asarray(inputs["q_inputs"], np.float32)[0]          # [QL, D]
    bias = np.asarray(inputs["bias"], np.float32)[0]           # [H, QL, KVL]

    qiT = q[qs:qs + QS].T.reshape(2, 128, QS).transpose(1, 0, 2)

    b = bias[:, qs:qs + QS, :]                   # [H, QS, KVL]
    b = b.reshape(NG, HPG, QS, NKC, 128)         # [g, h', q, c, p]
    b = b.transpose(4, 0, 3, 1, 2)               # [p, g, c, h', q]
    bT = b.reshape(128, NG, NKC, HPG * QS)       # [128, 2, 16, 1024]

    # ACT-path chunks carry fp8 bias (consumed by an fp8 identity-matmul
    # inject); DVE-path chunks carry the Schraudolph premultiplied int16
    # bias, consumed directly by the fused scalar_tensor_tensor exp.
    fp8_cs = [c for c in range(NKC) if c % 4 != 3]
    dve_cs = [c for c in range(NKC) if c % 4 == 3]
    b8 = np.ascontiguousarray(bT[:, :, fp8_cs]).astype(F8)
    bs = np.rint(A16 * bT[:, :, dve_cs] + B16).astype(np.int16)

    return {
        "qiT": np.ascontiguousarray(qiT).astype(BF16),
        "b8": b8,
        "bs": np.ascontiguousarray(bs),
    }


def make_in_maps(inputs):
    shared = _pack_shared(inputs)
    maps = []
    for core in range(NCORES):
        m = dict(shared)
        m.update(_pack_core(inputs, core))
        maps.append(m)
    return maps


def gather_output(results):
    out = np.empty((1, QL, O), np.float32)
    for core, res in enumerate(results):
        oT = np.asarray(res["out"], np.float32).reshape(O, QS)  # [o, q]
        out[0, core * QS:(core + 1) * QS, :] = oT.T
    return out


# ---------------------------------------------------------------------------
# Numpy mimic of the device dataflow (1:1 with the device matmuls) for
# validating the packing / orientation algebra without hardware.
# ---------------------------------------------------------------------------

def _bf(x):
    return x.astype(BF16).astype(np.float32)


def numpy_model(inputs):
    maps = make_in_maps(inputs)
    results = []
    for core in range(NCORES):
        m = {k: np.asarray(v, np.float32) for k, v in maps[core].items()
             if k != "bs"}
        kviT, qiT = m["kviT"], m["qiT"]
        b8 = m["b8"]                             # fp8 bias (ACT chunks)
        bs = np.asarray(maps[core]["bs"], np.float32)  # premult (DVE chunks)
        wka, wqg, wob, wpk32 = m["wka"], m["wqg"], m["wob"], m["wpk32"]
        wk = wka[:, 0:512].reshape(128, 2, 2, 128)
        wv = wka[:, 512:1024].reshape(128, 2, 256)
        iden = wka[:, 1024:1152]
        wqp = wqg[:, 0:2048].reshape(128, H, 2, 128)
        wgp = wqg[:, 2048:3072].reshape(128, NG * 2, 2, 128)
        ow = wob[:, 0:1024].reshape(128, NG * 2, 2, 128)
        ind2 = wob[:, 1024:1152]
        qbp = wpk32[:, 0:8]
        gbn = wpk32[:, 8:12]
        kb = wpk32[:, 12:14]
        vbb = wpk32[:, 14:270]
        ob = wpk32[:, 270:272]

        qTp = np.zeros((128, H, QS), np.float32)
        for h in range(H):
            acc = np.zeros((128, QS), np.float32)
            for kc in range(2):
                acc += wqp[:, h, kc, :].T @ qiT[:, kc, :]
            qTp[:, h, :] = _bf(acc + qbp[:, h:h + 1])

        sigT = np.zeros((128, NG * 2, QS), np.float32)
        for gb in range(NG * 2):
            acc = np.zeros((128, QS), np.float32)
            for kc in range(2):
                acc += wgp[:, gb, kc, :].T @ qiT[:, kc, :]
            sigT[:, gb, :] = 0.5 * np.tanh(0.5 * acc + gbn[:, gb:gb + 1]) + 0.5

        kT = np.zeros((128, NG, KVL), np.float32)
        for t in range(NG):
            acc = np.zeros((128, KVL), np.float32)
            for kc in range(2):
                acc += wk[:, t, kc, :].T @ kviT[:, kc, :]
            kT[:, t, :] = _bf(acc + kb[:, t:t + 1])

        vt = np.zeros((128, NKC, H, 33), np.float32)
        vt[:, :, :, 32] = 1.0
        for c in range(NKC):
            acc = np.zeros((128, H * C), np.float32)
            for kc in range(2):
                acc += kviT[:, kc, c * 128:(c + 1) * 128].T @ wv[:, kc, :]
            vt[:, c, :, :32] = _bf(acc + vbb).reshape(128, H, C)

        agT = np.zeros((128, NG * 2, QS), np.float32)
        for g in range(NG):
            accb = [np.zeros((128, 512), np.float32) for _ in range(2)]
            for c in range(NKC):
                lt = np.zeros((128, HPG, QS), np.float32)
                for b2 in range(2):
                    if not _dve_exp(g, c, b2):
                        lt[:, 2 * b2:2 * b2 + 2, :] += \
                            b8[:, g, _b8_idx(c),
                               512 * b2:512 * (b2 + 1)].reshape(128, 2, QS)
                for hp in range(HPG):
                    h = HPG * g + hp
                    lt[:, hp, :] += kT[:, g, c * 128:(c + 1) * 128].T @ qTp[:, h, :]
                et = np.zeros((128, HPG, QS), np.float32)
                for b2 in range(2):
                    sl = lt[:, 2 * b2:2 * b2 + 2, :]
                    if _dve_exp(g, c, b2):
                        bsl = bs[:, g, c // 4, 512 * b2:512 * (b2 + 1)]
                        iv = np.rint(A16 * sl + bsl.reshape(128, 2, QS))
                        et[:, 2 * b2:2 * b2 + 2, :] = \
                            iv.astype(np.int16).view(BF16).astype(np.float32)
                    else:
                        et[:, 2 * b2:2 * b2 + 2, :] = _bf(np.exp(sl))
                for hp in range(HPG):
                    h = HPG * g + hp
                    b2, j = hp // 2, hp % 2
                    accb[b2][64 * j:64 * j + 33, 0:QS] += \
                        vt[:, c, h, :].T @ et[:, hp, :]
            for b2 in range(2):
                rsg = np.zeros((128, QS), np.float32)
                rsg[32] = _bf(accb[b2][32, 0:QS])
                rsg[96] = _bf(accb[b2][96, 0:QS])
                rsb = ind2.T @ rsg
                recipB = 1.0 / rsb
                gb = 2 * g + b2
                agT[:, gb, :] = _bf(accb[b2][:, 0:QS] * sigT[:, gb, :] * recipB)

        outT = np.zeros((2, 128, QS), np.float32)
        for t in range(2):
            acc = np.zeros((128, QS), np.float32)
            for gb in range(NG * 2):
                acc += ow[:, gb, t, :].T @ agT[:, gb, :]
            outT[t] = acc + ob[:, t:t + 1]
        results.append({"out": outT})
    return gather_output(results)


# ---------------------------------------------------------------------------
# Device kernel builder
# ---------------------------------------------------------------------------

def build_kernel():
    nc = bacc.Bacc("TRN2", target_bir_lowering=False, debug=False)

    p_wka = nc.declare_dram_parameter("wka", [128, 1152], bf16, False)
    p_wqg = nc.declare_dram_parameter("wqg", [128, 3072], bf16, False)
    p_wob = nc.declare_dram_parameter("wob", [128, 1152], bf16, False)
    p_wpk32 = nc.declare_dram_parameter("wpk32", [128, 272], f32, False)
    p_i8 = nc.declare_dram_parameter("i8", [128, 128], f8e4, False)
    p_qiT = nc.declare_dram_parameter("qiT", [128, 2, QS], bf16, False)
    p_kviT = nc.declare_dram_parameter("kviT", [128, 2, KVL], bf16, False)
    p_b8 = nc.declare_dram_parameter("b8", [128, NG, 12, HPG * QS], f8e4, False)
    p_bs = nc.declare_dram_parameter("bs", [128, NG, 4, HPG * QS], i16, False)
    p_out = nc.declare_dram_parameter("out", [2, 128, QS], f32, True)

    Exp = mybir.ActivationFunctionType.Exp
    Tanh = mybir.ActivationFunctionType.Tanh
    ADD = mybir.AluOpType.add
    MUL = mybir.AluOpType.mult

    with tile.TileContext(nc) as tc:
        with (
            tc.tile_pool(name="sb", bufs=1) as sb,
            tc.tile_pool(name="etp", bufs=3) as etp,
            tc.tile_pool(name="tmp", bufs=2) as tmp,
            tc.tile_pool(name="ps", bufs=2, space="PSUM") as ps,
            tc.tile_pool(name="pswork", bufs=2, space="PSUM") as pswork,
        ):
            # ---- resident SBUF loads, ordered by first consumption ----
            s_wpk32 = sb.tile([128, 272], f32)
            nc.sync.dma_start(out=s_wpk32, in_=p_wpk32[:])
            s_wka = sb.tile([128, 1152], bf16)
            nc.sync.dma_start(out=s_wka, in_=p_wka[:])
            s_i8 = sb.tile([128, 128], f8e4)
            nc.sync.dma_start(out=s_i8, in_=p_i8[:])
            s_kviT = sb.tile([128, 2, KVL], bf16)
            nc.sync.dma_start(out=s_kviT, in_=p_kviT[:])
            s_qiT = sb.tile([128, 2, QS], bf16)
            nc.sync.dma_start(out=s_qiT, in_=p_qiT[:])
            s_wqg = sb.tile([128, 3072], bf16)
            nc.sync.dma_start(out=s_wqg, in_=p_wqg[:])

            # bias streams: fp8 (ACT chunks) + int16 premult (DVE chunks),
            # group 0 first, halves interleaved by consumption order.
            s_b8 = sb.tile([128, NG, 12, HPG * QS], f8e4)
            s_bs = sb.tile([128, NG, 4, HPG * QS], i16)
            s_wob = sb.tile([128, 1152], bf16)
            for g in range(NG):
                nc.sync.dma_start(out=s_b8[:, g, 0:6, :], in_=p_b8[:, g, 0:6, :])
                nc.sync.dma_start(out=s_bs[:, g, 0:2, :], in_=p_bs[:, g, 0:2, :])
                nc.sync.dma_start(out=s_b8[:, g, 6:12, :], in_=p_b8[:, g, 6:12, :])
                nc.sync.dma_start(out=s_bs[:, g, 2:4, :], in_=p_bs[:, g, 2:4, :])
                if g == 0:
                    nc.sync.dma_start(out=s_wob, in_=p_wob[:])

            s_wk = s_wka[:, 0:512].rearrange("p (t k m) -> p t k m", t=2, k=2)
            s_wv = s_wka[:, 512:1024].rearrange("p (k m) -> p k m", k=2)
            s_iden = s_wka[:, 1024:1152]
            s_wqp = s_wqg[:, 0:2048].rearrange("p (h k m) -> p h k m", h=H, k=2)
            s_wgp = s_wqg[:, 2048:3072].rearrange("p (g k m) -> p g k m", g=NG * 2, k=2)
            s_ow = s_wob[:, 0:1024].rearrange("p (g t m) -> p g t m", g=NG * 2, t=2)
            s_ind2 = s_wob[:, 1024:1152]
            s_qbp = s_wpk32[:, 0:8]
            s_gbn = s_wpk32[:, 8:12]
            s_kb = s_wpk32[:, 12:14]
            s_vbb = s_wpk32[:, 14:270]
            s_ob = s_wpk32[:, 270:272]

            s_zcol = sb.tile([1, 128], bf16)
            nc.vector.memset(s_zcol, 0.0)
            s_zrow = sb.tile([1, 512], bf16)
            nc.vector.memset(s_zrow, 0.0)

            # ---- kT projection (bf16, packed 4 heads / tile) ----
            # ns-outer so kT chunks 0-3 are ready as soon as the first kviT
            # quarter lands, unblocking the attention pipeline early.
            s_kT = sb.tile([128, 2, KVL], bf16)
            for ns in range(4):
                for t in range(2):
                    pt = pswork.tile([128, 512], f32, tag="work", name=f"kt_ps_{t}_{ns}")
                    for kc in range(2):
                        nc.tensor.matmul(
                            pt, lhsT=s_wk[:, t, kc, :],
                            rhs=s_kviT[:, kc, ns * 512:(ns + 1) * 512],
                            start=(kc == 0), stop=(kc == 1),
                        )
                    nc.vector.tensor_scalar_add(
                        s_kT[:, t, ns * 512:(ns + 1) * 512], pt, s_kb[:, t:t + 1])

            # ---- qg projection -> per-head padded qT (bf16), sigT (f32) ----
            s_qT = sb.tile([128, H, QS], bf16)
            s_sigT = sb.tile([128, NG * 2, QS], f32)
            for h in range(H):
                pt = pswork.tile([128, 512], f32, tag="work", name=f"q_ps_{h}")
                for kc in range(2):
                    nc.tensor.matmul(
                        pt[:, :QS], lhsT=s_wqp[:, h, kc, :], rhs=s_qiT[:, kc, :],
                        start=(kc == 0), stop=(kc == 1),
                    )
                nc.vector.tensor_scalar_add(s_qT[:, h, :], pt[:, :QS], s_qbp[:, h:h + 1])
            for gb in range(NG * 2):
                pt = pswork.tile([128, 512], f32, tag="work", name=f"g_ps_{gb}")
                for kc in range(2):
                    nc.tensor.matmul(
                        pt[:, :QS], lhsT=s_wgp[:, gb, kc, :], rhs=s_qiT[:, kc, :],
                        start=(kc == 0), stop=(kc == 1),
                    )
                # sigma(x) = 0.5*tanh(x/2) + 0.5; tanh shares the Exp table set
                t_u = tmp.tile([128, QS], f32, tag="sigtmp", name=f"sig_u_{gb}")
                nc.scalar.activation(t_u, pt[:, :QS], Tanh,
                                     bias=s_gbn[:, gb:gb + 1], scale=0.5)
                nc.vector.tensor_scalar(s_sigT[:, gb, :], t_u, 0.5, 0.5,
                                        mybir.AluOpType.mult, mybir.AluOpType.add)

            # ---- v projection with ones column (bf16) ----
            s_v = sb.tile([128, NKC, H, 33], bf16)
            nc.vector.memset(s_v[:, :, :, 32:33], 1.0)
            for c in range(NKC):
                pt = pswork.tile([128, 512], f32, tag="work", name=f"v_ps_{c}")
                for kc in range(2):
                    nc.tensor.matmul(
                        pt[:, :256], lhsT=s_kviT[:, kc, c * 128:(c + 1) * 128],
                        rhs=s_wv[:, kc, :],
                        start=(kc == 0), stop=(kc == 1),
                    )
                nc.vector.tensor_tensor(
                    s_v[:, c, :, 0:32],
                    pt[:, :256].rearrange("p (h x) -> p h x", h=H),
                    s_vbb.rearrange("p (h x) -> p h x", h=H), ADD)

            # ---- attention, one head-group (4 heads = 2 banks) at a time ----
            s_agT = sb.tile([128, NG * 2, QS], bf16)
            for g in range(NG):
                accs = []
                for b2 in range(2):
                    acc = ps.tile([128, 512], f32, tag="accum", name=f"acc_{g}_{b2}")
                    nc.tensor.matmul(acc, lhsT=s_zcol, rhs=s_zrow, start=True,
                                     stop=False, skip_group_check=True)
                    accs.append(acc)
                for c in range(NKC):
                    lt = ps.tile([128, HPG, QS], f32, tag="lt", name=f"lt_{g}_{c}")
                    # Per-head K=64 row-band matmuls via 2x row tiling: band-0
                    # (T0) and band-64 (T8) matmuls execute concurrently in
                    # the PE array and write different PSUM banks.  The q
                    # panes are zero outside each head's 32 rows, so the
                    # extra 32 contraction rows of the band are inert.
                    for j in range(2):
                        for bd in range(2):
                            hp = 2 * bd + j          # head hp on band 64*bd
                            h = HPG * g + hp
                            p0 = 64 * bd
                            # bank index == band index here (heads 2bd, 2bd+1)
                            last = (j == 1) and _dve_exp(g, c, bd)
                            nc.tensor.matmul(
                                lt[:, hp, :],
                                lhsT=s_kT[p0:p0 + 64, g, c * 128:(c + 1) * 128],
                                rhs=s_qT[p0:p0 + 64, h, :],
                                start=(j == 0), stop=last,
                                tile_position=(p0, 0),
                                skip_group_check=True,
                            )
                    for b2 in range(2):
                        if not _dve_exp(g, c, b2):
                            nc.tensor.matmul(
                                lt[:, 2 * b2:2 * b2 + 2, :], lhsT=s_i8,
                                rhs=s_b8[:, g, _b8_idx(c),
                                         512 * b2:512 * (b2 + 1)],
                                start=False, stop=True, skip_group_check=True,
                            )
                    et = etp.tile([128, HPG, QS], bf16, tag="et", name=f"et_{g}_{c}")
                    for b2 in range(2):  # ACT must not cross PSUM banks
                        if _dve_exp(g, c, b2):
                            # exp(logits+bias) in one DVE pass: Schraudolph
                            # int16 bitcast with host-premultiplied bias.
                            nc.vector.scalar_tensor_tensor(
                                out=et[:, 2 * b2:2 * b2 + 2, :].bitcast(i16),
                                in0=lt[:, 2 * b2:2 * b2 + 2, :],
                                scalar=A16,
                                in1=s_bs[:, g, c // 4, 512 * b2:512 * (b2 + 1)]
                                    .rearrange("p (h q) -> p h q", h=2),
                                op0=MUL, op1=ADD,
                            )
                        else:
                            nc.scalar.activation(et[:, 2 * b2:2 * b2 + 2, :],
                                                 lt[:, 2 * b2:2 * b2 + 2, :], Exp)
                    for hp in range(HPG):
                        h = HPG * g + hp
                        b2, j = hp // 2, hp % 2
                        nc.tensor.matmul(
                            accs[b2][64 * j:64 * j + 33, 0:QS],
                            lhsT=s_v[:, c, h, :], rhs=et[:, hp, :],
                            start=False, stop=(c == NKC - 1),
                            tile_position=(0, 64 * j), skip_group_check=True,
                        )
                # softmax denominator + gating, per bank
                for b2 in range(2):
                    gb = 2 * g + b2
                    acc = accs[b2]
                    rsg = tmp.tile([128, QS], bf16, tag="rsg", name=f"rsg_{gb}")
                    nc.vector.memset(rsg, 0.0)
                    nc.vector.tensor_copy(out=rsg[32:33, :], in_=acc[32:33, 0:QS])
                    nc.vector.tensor_copy(out=rsg[96:97, :], in_=acc[96:97, 0:QS])
                    rsb = pswork.tile([128, 512], f32, tag="work", name=f"rsb_{gb}")
                    nc.tensor.matmul(rsb[:, :QS], lhsT=s_ind2, rhs=rsg,
                                     start=True, stop=True)
                    recipB = tmp.tile([128, QS], f32, tag="recip", name=f"recip_{gb}")
                    nc.vector.reciprocal_approx_fast(out=recipB, in_=rsb[:, :QS])
                    gt1 = tmp.tile([128, QS], f32, tag="gt1", name=f"gt1_{gb}")
                    nc.vector.tensor_tensor(gt1, acc[:, 0:QS], s_sigT[:, gb, :], MUL)
                    nc.vector.tensor_tensor(s_agT[:, gb, :], gt1, recipB, MUL)

            # ---- output projection ----
            s_outT = sb.tile([128, 2, QS], f32)
            for t in range(2):
                pt = pswork.tile([128, 512], f32, tag="work", name=f"o_ps_{t}")
                for gb in range(NG * 2):
                    nc.tensor.matmul(
                        pt[:, :QS], lhsT=s_ow[:, gb, t, :], rhs=s_agT[:, gb, :],
                        start=(gb == 0), stop=(gb == NG * 2 - 1),
                    )
                nc.scalar.add(s_outT[:, t, :], pt[:, :QS], s_ob[:, t:t + 1])
                nc.sync.dma_start(out=p_out[t], in_=s_outT[:, t, :])

    nc.finalize()
    return nc


_NC = None


def _get_nc():
    global _NC
    if _NC is None:
        _NC = build_kernel()
    return _NC


def kernel(**inputs) -> np.ndarray:
    nc = _get_nc()
    in_maps = make_in_maps(inputs)
    res = run_bass_kernel_spmd(nc, in_maps, core_ids=list(range(NCORES)))
    return gather_output(res.results)


def kernel_traced(**inputs):
    """Like kernel() but with NTFF profiling; returns (output, exec_time_ns, res)."""
    nc = _get_nc()
    in_maps = make_in_maps(inputs)
    res = run_bass_kernel_spmd(nc, in_maps, core_ids=list(range(NCORES)), trace=True)
    return gather_output(res.results), res.exec_time_ns, res



# revision 49
# speedup vs baseline: 1.1597x; 1.1597x over previous
"""Trainium2 Bass kernel for nn_Attention_73486890434886.

Gated 8-head attention (head_dim 32) with a full [8, 2048, 2048] attention
bias, batch 1, q_len = kv_len = 2048, fused QG / KV projections and a gated
output projection.

Strategy (8 NeuronCores, SPMD, no collectives):
  - Shard the 2048 q rows across the 8 cores (256 rows each).  Every core
    computes all 8 heads for its q-slice; kv-side projections are replicated
    (cheap), which removes the output all-reduce entirely.
  - All attention math is in a "transposed" orientation so no on-device
    transposes are needed: logits^T [kv, q] come from k-stationary x
    (zero-padded per-head) q-moving matmuls, the host-pretransposed bias is
    injected into PSUM with an identity-stationary matmul (or a DVE add),
    exp runs on the scalar engine, and attn@v consumes exp(logits^T) as the
    matmul moving operand producing attn_out^T [c, q].  Softmax denominators
    ride along as a ones-column appended to the v stationary (M=33).
  - bf16 on the TensorEngine (fp32 PSUM accumulation), f32 softmax on ACT.
"""

import numpy as np
import ml_dtypes

import concourse.bass as bass
import concourse.mybir as mybir
import concourse.tile as tile
from concourse import bacc
from concourse.bass_utils import run_bass_kernel_spmd

BF16 = ml_dtypes.bfloat16

# Problem shapes (hardcoded per the task statement).
B, QL, KVL, D, H, C, O = 1, 2048, 2048, 256, 8, 32, 256
NCORES = 8
QS = QL // NCORES          # 256 q rows per core
NKC = KVL // 128           # 16 kv chunks of 128
NG = 2                     # head groups (0-3, 4-7)
HPG = H // NG              # heads per group = 4

# Head-pair banks: group g, bank b -> heads (4g+2b, 4g+2b+1); within an acc
# bank the two heads sit at partitions 0..32 and 64..96 (numer rows +
# trailing rowsum row from the ones column of the v stationary).

f32 = mybir.dt.float32
bf16 = mybir.dt.bfloat16

# Banks whose bias-add runs on DVE instead of the TensorEngine (load balance).
def _dve_inject(g, c, b):
    return False


# ---------------------------------------------------------------------------
# Host-side packing: everything is laid out partition-major so every DMA is a
# straight contiguous copy.
# ---------------------------------------------------------------------------

def _pack_shared(inputs):
    kv = np.asarray(inputs["kv_inputs"], np.float32)[0]        # [KVL, D]
    qg_w = np.asarray(inputs["qg_weights"], np.float32)[:, 0]  # [D, H, 2C]
    qg_b = np.asarray(inputs["qg_bias"], np.float32)[0, :, 0]  # [H, 2C]
    kv_w = np.asarray(inputs["kv_weights"], np.float32)[:, 0]  # [D, H, 2C]
    kv_b = np.asarray(inputs["kv_bias"], np.float32)[0, :, 0]  # [H, 2C]
    o_w = np.asarray(inputs["o_weights"], np.float32)[0]       # [H, C, O]
    o_b = np.asarray(inputs["o_bias"], np.float32)[:, 0]       # [O]

    scale = C ** -0.5

    # Per-head zero-padded q weights: stationary tile h has w_q in column
    # block 32h'..32h'+32, zeros elsewhere, so the logits matmul can contract
    # over the full 128 partitions of the packed k tile without mixing heads.
    wq_full = qg_w[:, :, :C] * scale           # [D, H, C]
    wq_pad = np.zeros((D, H, 128), np.float32)
    for h in range(H):
        hp = h % HPG
        wq_pad[:, h, 32 * hp:32 * hp + 32] = wq_full[:, h, :]
    wq_pad = wq_pad.reshape(2, 128, H, 128).transpose(1, 2, 0, 3)  # [128,H,kc,128]

    # Gate weights in head-pair "bank" layout: tile (g,b) has head 4g+2b at
    # columns 0..32 and head 4g+2b+1 at columns 64..96, zeros elsewhere.
    wg_full = qg_w[:, :, C:]                   # [D, H, C]
    wg_pair = np.zeros((D, NG * 2, 128), np.float32)
    gbn = np.zeros((128, NG * 2), np.float32)  # gate_bias / 2, same layout
    for g in range(NG):
        for b in range(2):
            for j in range(2):
                h = 4 * g + 2 * b + j
                wg_pair[:, 2 * g + b, 64 * j:64 * j + C] = wg_full[:, h, :]
                gbn[64 * j:64 * j + C, 2 * g + b] = 0.5 * qg_b[h, C:]
    wg_pair = wg_pair.reshape(2, 128, NG * 2, 128).transpose(1, 2, 0, 3)

    # Packed k weights: [128, NG, kc, 128] with m = h'*C + c.
    wk = kv_w[:, :, :C].reshape(D, NG, HPG * C)
    wk = wk.transpose(1, 0, 2).reshape(NG, 2, 128, HPG * C).transpose(2, 0, 1, 3)

    wv = kv_w[:, :, C:].reshape(D, H * C)
    wv = wv.reshape(2, 128, H * C).transpose(1, 0, 2)          # [128, 2, 256]

    qb_full = qg_b[:, :C] * scale
    qbp = np.zeros((128, H), np.float32)
    for h in range(H):
        hp = h % HPG
        qbp[32 * hp:32 * hp + 32, h] = qb_full[h]
    kb = kv_b[:, :C].reshape(NG, 128).T                        # [128, 2]
    vbb = np.broadcast_to(kv_b[:, C:].reshape(1, H * C), (128, H * C)).copy()

    # o weights in bank layout with zero rows outside the two 32-row head
    # blocks (kills the junk rows of the gated-attention tile).
    ow = np.zeros((128, NG * 2, 2, 128), np.float32)
    o_flat = o_w.reshape(H * C, O)             # [(h,c), o]
    for g in range(NG):
        for b in range(2):
            for j in range(2):
                h = 4 * g + 2 * b + j
                for t in range(2):
                    ow[64 * j:64 * j + C, 2 * g + b, t, :] = \
                        o_flat[h * C:(h + 1) * C, t * 128:(t + 1) * 128]
    ob = o_b.reshape(2, 128).T                 # [128, 2]

    kviT = kv.T.reshape(2, 128, KVL).transpose(1, 0, 2)        # [128, 2, KVL]

    iden = np.eye(128, dtype=np.float32)
    ind2 = np.zeros((128, 128), np.float32)    # row broadcast: m <- 64*(m//64)+32
    for m in range(128):
        ind2[64 * (m // 64) + 32, m] = 1.0

    # One bf16 pack and one f32 pack so startup is 2 DMAs, not 13.
    wpk = np.concatenate([
        wq_pad.reshape(128, -1), wg_pair.reshape(128, -1), wk.reshape(128, -1),
        wv.reshape(128, -1), ow.reshape(128, -1), iden, ind2,
    ], axis=1)                                  # [128, 5376]
    wpk32 = np.concatenate([qbp, gbn, kb, vbb, ob], axis=1)  # [128, 272]
    return {
        "kviT": kviT.astype(BF16),
        "wpk": np.ascontiguousarray(wpk).astype(BF16),
        "wpk32": np.ascontiguousarray(wpk32).astype(np.float32),
    }


def _pack_core(inputs, core):
    qs = core * QS
    q = np.asarray(inputs["q_inputs"], np.float32)[0]          # [QL, D]
    bias = np.asarray(inputs["bias"], np.float32)[0]           # [H, QL, KVL]

    qiT = q[qs:qs + QS].T.reshape(2, 128, QS).transpose(1, 0, 2)

    b = bias[:, qs:qs + QS, :]                   # [H, QS, KVL]
    b = b.reshape(NG, HPG, QS, NKC, 128)         # [g, h', q, c, p]
    b = b.transpose(4, 0, 3, 1, 2)               # [p, g, c, h', q]
    bT = b.reshape(128, NG, NKC, HPG * QS)       # [128, 2, 16, 1024]

    return {
        "qiT": np.ascontiguousarray(qiT).astype(BF16),
        "bT": np.ascontiguousarray(bT).astype(BF16),
    }


def make_in_maps(inputs):
    shared = _pack_shared(inputs)
    maps = []
    for core in range(NCORES):
        m = dict(shared)
        m.update(_pack_core(inputs, core))
        maps.append(m)
    return maps


def gather_output(results):
    out = np.empty((1, QL, O), np.float32)
    for core, res in enumerate(results):
        oT = np.asarray(res["out"], np.float32).reshape(O, QS)  # [o, q]
        out[0, core * QS:(core + 1) * QS, :] = oT.T
    return out


# ---------------------------------------------------------------------------
# Numpy mimic of the device dataflow (1:1 with the device matmuls) for
# validating the packing / orientation algebra without hardware.
# ---------------------------------------------------------------------------

def _bf(x):
    return x.astype(BF16).astype(np.float32)


def numpy_model(inputs):
    maps = make_in_maps(inputs)
    results = []
    for core in range(NCORES):
        m = {k: np.asarray(v, np.float32) for k, v in maps[core].items()}
        kviT, qiT, bT = m["kviT"], m["qiT"], m["bT"]
        wpk, wpk32 = m["wpk"], m["wpk32"]
        wqp = wpk[:, 0:2048].reshape(128, H, 2, 128)
        wgp = wpk[:, 2048:3072].reshape(128, NG * 2, 2, 128)
        wk = wpk[:, 3072:3584].reshape(128, 2, 2, 128)
        wv = wpk[:, 3584:4096].reshape(128, 2, 256)
        ow = wpk[:, 4096:5120].reshape(128, NG * 2, 2, 128)
        iden = wpk[:, 5120:5248]
        ind2 = wpk[:, 5248:5376]
        qbp = wpk32[:, 0:8]
        gbn = wpk32[:, 8:12]
        kb = wpk32[:, 12:14]
        vbb = wpk32[:, 14:270]
        ob = wpk32[:, 270:272]

        qTp = np.zeros((128, H, QS), np.float32)
        for h in range(H):
            acc = np.zeros((128, QS), np.float32)
            for kc in range(2):
                acc += wqp[:, h, kc, :].T @ qiT[:, kc, :]
            qTp[:, h, :] = _bf(acc + qbp[:, h:h + 1])

        sigT = np.zeros((128, NG * 2, QS), np.float32)
        for gb in range(NG * 2):
            acc = np.zeros((128, QS), np.float32)
            for kc in range(2):
                acc += wgp[:, gb, kc, :].T @ qiT[:, kc, :]
            sigT[:, gb, :] = 0.5 * np.tanh(0.5 * acc + gbn[:, gb:gb + 1]) + 0.5

        kT = np.zeros((128, NG, KVL), np.float32)
        for t in range(NG):
            acc = np.zeros((128, KVL), np.float32)
            for kc in range(2):
                acc += wk[:, t, kc, :].T @ kviT[:, kc, :]
            kT[:, t, :] = _bf(acc + kb[:, t:t + 1])

        vt = np.zeros((128, NKC, H, 33), np.float32)
        vt[:, :, :, 32] = 1.0
        for c in range(NKC):
            acc = np.zeros((128, H * C), np.float32)
            for kc in range(2):
                acc += kviT[:, kc, c * 128:(c + 1) * 128].T @ wv[:, kc, :]
            vt[:, c, :, :32] = _bf(acc + vbb).reshape(128, H, C)

        agT = np.zeros((128, NG * 2, QS), np.float32)
        for g in range(NG):
            accb = [np.zeros((128, 512), np.float32) for _ in range(2)]
            for c in range(NKC):
                lt = np.zeros((128, HPG, QS), np.float32)
                for b2 in range(2):
                    lt[:, 2 * b2:2 * b2 + 2, :] += \
                        bT[:, g, c, 512 * b2:512 * (b2 + 1)].reshape(128, 2, QS)
                for hp in range(HPG):
                    h = HPG * g + hp
                    lt[:, hp, :] += kT[:, g, c * 128:(c + 1) * 128].T @ qTp[:, h, :]
                et = _bf(np.exp(lt))
                for hp in range(HPG):
                    h = HPG * g + hp
                    b2, j = hp // 2, hp % 2
                    accb[b2][64 * j:64 * j + 33, 0:QS] += \
                        vt[:, c, h, :].T @ et[:, hp, :]
            for b2 in range(2):
                rsg = np.zeros((128, QS), np.float32)
                rsg[32] = _bf(accb[b2][32, 0:QS])
                rsg[96] = _bf(accb[b2][96, 0:QS])
                rsb = ind2.T @ rsg
                recipB = 1.0 / rsb
                gb = 2 * g + b2
                agT[:, gb, :] = _bf(accb[b2][:, 0:QS] * sigT[:, gb, :] * recipB)

        outT = np.zeros((2, 128, QS), np.float32)
        for t in range(2):
            acc = np.zeros((128, QS), np.float32)
            for gb in range(NG * 2):
                acc += ow[:, gb, t, :].T @ agT[:, gb, :]
            outT[t] = acc + ob[:, t:t + 1]
        results.append({"out": outT})
    return gather_output(results)


# ---------------------------------------------------------------------------
# Device kernel builder
# ---------------------------------------------------------------------------

def build_kernel():
    nc = bacc.Bacc("TRN2", target_bir_lowering=False, debug=False)

    p_wpk = nc.declare_dram_parameter("wpk", [128, 5376], bf16, False)
    p_wpk32 = nc.declare_dram_parameter("wpk32", [128, 272], f32, False)
    p_qiT = nc.declare_dram_parameter("qiT", [128, 2, QS], bf16, False)
    p_kviT = nc.declare_dram_parameter("kviT", [128, 2, KVL], bf16, False)
    p_bT = nc.declare_dram_parameter("bT", [128, NG, NKC, HPG * QS], bf16, False)
    p_out = nc.declare_dram_parameter("out", [2, 128, QS], f32, True)

    Exp = mybir.ActivationFunctionType.Exp
    Tanh = mybir.ActivationFunctionType.Tanh
    ADD = mybir.AluOpType.add
    MUL = mybir.AluOpType.mult

    with tile.TileContext(nc) as tc:
        with (
            tc.tile_pool(name="sb", bufs=1) as sb,
            tc.tile_pool(name="etp", bufs=3) as etp,
            tc.tile_pool(name="tmp", bufs=2) as tmp,
            tc.tile_pool(name="ps", bufs=2, space="PSUM") as ps,
            tc.tile_pool(name="pswork", bufs=2, space="PSUM") as pswork,
        ):
            # ---- resident SBUF loads: packed DMAs + inputs ----
            s_qiT = sb.tile([128, 2, QS], bf16)
            nc.sync.dma_start(out=s_qiT, in_=p_qiT[:])
            s_wpk32 = sb.tile([128, 272], f32)
            nc.sync.dma_start(out=s_wpk32, in_=p_wpk32[:])
            s_wpk = sb.tile([128, 5376], bf16)
            nc.sync.dma_start(out=s_wpk, in_=p_wpk[:])
            s_kviT = sb.tile([128, 2, KVL], bf16)
            nc.sync.dma_start(out=s_kviT, in_=p_kviT[:])
            s_wqp = s_wpk[:, 0:2048].rearrange("p (h k m) -> p h k m", h=H, k=2)
            s_wgp = s_wpk[:, 2048:3072].rearrange("p (g k m) -> p g k m", g=NG * 2, k=2)
            s_wk = s_wpk[:, 3072:3584].rearrange("p (t k m) -> p t k m", t=2, k=2)
            s_wv = s_wpk[:, 3584:4096].rearrange("p (k m) -> p k m", k=2)
            s_ow = s_wpk[:, 4096:5120].rearrange("p (g t m) -> p g t m", g=NG * 2, t=2)
            s_iden = s_wpk[:, 5120:5248]
            s_ind2 = s_wpk[:, 5248:5376]
            s_qbp = s_wpk32[:, 0:8]
            s_gbn = s_wpk32[:, 8:12]
            s_kb = s_wpk32[:, 12:14]
            s_vbb = s_wpk32[:, 14:270]
            s_ob = s_wpk32[:, 270:272]

            s_zcol = sb.tile([1, 128], bf16)
            nc.vector.memset(s_zcol, 0.0)
            s_zrow = sb.tile([1, 512], bf16)
            nc.vector.memset(s_zrow, 0.0)


            # bias, streamed in 8 chunks ordered by consumption
            s_bT = sb.tile([128, NG, NKC, HPG * QS], bf16)
            for g in range(NG):
                for qtr in range(4):
                    c0 = qtr * (NKC // 4)
                    nc.sync.dma_start(
                        out=s_bT[:, g, c0:c0 + NKC // 4, :],
                        in_=p_bT[:, g, c0:c0 + NKC // 4, :],
                    )

            # ---- qg projection -> per-head padded qT (bf16), sigT (f32) ----
            s_qT = sb.tile([128, H, QS], bf16)
            s_sigT = sb.tile([128, NG * 2, QS], f32)
            for h in range(H):
                pt = pswork.tile([128, 512], f32, tag="work", name=f"q_ps_{h}")
                for kc in range(2):
                    nc.tensor.matmul(
                        pt[:, :QS], lhsT=s_wqp[:, h, kc, :], rhs=s_qiT[:, kc, :],
                        start=(kc == 0), stop=(kc == 1),
                    )
                nc.vector.tensor_scalar_add(s_qT[:, h, :], pt[:, :QS], s_qbp[:, h:h + 1])
            for gb in range(NG * 2):
                pt = pswork.tile([128, 512], f32, tag="work", name=f"g_ps_{gb}")
                for kc in range(2):
                    nc.tensor.matmul(
                        pt[:, :QS], lhsT=s_wgp[:, gb, kc, :], rhs=s_qiT[:, kc, :],
                        start=(kc == 0), stop=(kc == 1),
                    )
                # sigma(x) = 0.5*tanh(x/2) + 0.5; tanh shares the Exp table set
                t_u = tmp.tile([128, QS], f32, tag="sigtmp", name=f"sig_u_{gb}")
                nc.scalar.activation(t_u, pt[:, :QS], Tanh,
                                     bias=s_gbn[:, gb:gb + 1], scale=0.5)
                nc.vector.tensor_scalar(s_sigT[:, gb, :], t_u, 0.5, 0.5,
                                        mybir.AluOpType.mult, mybir.AluOpType.add)

            # ---- kT projection (bf16, packed 4 heads / tile) ----
            s_kT = sb.tile([128, 2, KVL], bf16)
            for t in range(2):
                for ns in range(4):
                    pt = pswork.tile([128, 512], f32, tag="work", name=f"kt_ps_{t}_{ns}")
                    for kc in range(2):
                        nc.tensor.matmul(
                            pt, lhsT=s_wk[:, t, kc, :],
                            rhs=s_kviT[:, kc, ns * 512:(ns + 1) * 512],
                            start=(kc == 0), stop=(kc == 1),
                        )
                    nc.vector.tensor_scalar_add(
                        s_kT[:, t, ns * 512:(ns + 1) * 512], pt, s_kb[:, t:t + 1])

            # ---- v projection with ones column (bf16) ----
            s_v = sb.tile([128, NKC, H, 33], bf16)
            nc.vector.memset(s_v[:, :, :, 32:33], 1.0)
            for c in range(NKC):
                pt = pswork.tile([128, 512], f32, tag="work", name=f"v_ps_{c}")
                for kc in range(2):
                    nc.tensor.matmul(
                        pt[:, :256], lhsT=s_kviT[:, kc, c * 128:(c + 1) * 128],
                        rhs=s_wv[:, kc, :],
                        start=(kc == 0), stop=(kc == 1),
                    )
                nc.vector.tensor_tensor(
                    s_v[:, c, :, 0:32],
                    pt[:, :256].rearrange("p (h x) -> p h x", h=H),
                    s_vbb.rearrange("p (h x) -> p h x", h=H), ADD)

            # ---- attention, one head-group (4 heads = 2 banks) at a time ----
            s_agT = sb.tile([128, NG * 2, QS], bf16)
            for g in range(NG):
                accs = []
                for b2 in range(2):
                    acc = ps.tile([128, 512], f32, tag="accum", name=f"acc_{g}_{b2}")
                    nc.tensor.matmul(acc, lhsT=s_zcol, rhs=s_zrow, start=True,
                                     stop=False, skip_group_check=True)
                    accs.append(acc)
                for c in range(NKC):
                    lt = ps.tile([128, HPG, QS], f32, tag="lt", name=f"lt_{g}_{c}")
                    # Per-head K=64 row-band matmuls via 2x row tiling: band-0
                    # (T0) and band-64 (T8) matmuls execute concurrently in
                    # the PE array and write different PSUM banks.  The q
                    # panes are zero outside each head's 32 rows, so the
                    # extra 32 contraction rows of the band are inert.
                    for j in range(2):
                        for bd in range(2):
                            hp = 2 * bd + j          # head hp on band 64*bd
                            h = HPG * g + hp
                            p0 = 64 * bd
                            nc.tensor.matmul(
                                lt[:, hp, :],
                                lhsT=s_kT[p0:p0 + 64, g, c * 128:(c + 1) * 128],
                                rhs=s_qT[p0:p0 + 64, h, :],
                                start=(j == 0), stop=False,
                                tile_position=(p0, 0),
                                skip_group_check=True,
                            )
                    for b2 in range(2):
                        nc.tensor.matmul(
                            lt[:, 2 * b2:2 * b2 + 2, :], lhsT=s_iden,
                            rhs=s_bT[:, g, c, 512 * b2:512 * (b2 + 1)],
                            start=False, stop=True, skip_group_check=True,
                        )
                    et = etp.tile([128, HPG, QS], bf16, tag="et", name=f"et_{g}_{c}")
                    for b2 in range(2):  # ACT must not cross PSUM banks
                        nc.scalar.activation(et[:, 2 * b2:2 * b2 + 2, :],
                                             lt[:, 2 * b2:2 * b2 + 2, :], Exp)
                    for hp in range(HPG):
                        h = HPG * g + hp
                        b2, j = hp // 2, hp % 2
                        nc.tensor.matmul(
                            accs[b2][64 * j:64 * j + 33, 0:QS],
                            lhsT=s_v[:, c, h, :], rhs=et[:, hp, :],
                            start=False, stop=(c == NKC - 1),
                            tile_position=(0, 64 * j), skip_group_check=True,
                        )
                # softmax denominator + gating, per bank
                for b2 in range(2):
                    gb = 2 * g + b2
                    acc = accs[b2]
                    rsg = tmp.tile([128, QS], bf16, tag="rsg", name=f"rsg_{gb}")
                    nc.vector.memset(rsg, 0.0)
                    nc.vector.tensor_copy(out=rsg[32:33, :], in_=acc[32:33, 0:QS])
                    nc.vector.tensor_copy(out=rsg[96:97, :], in_=acc[96:97, 0:QS])
                    rsb = pswork.tile([128, 512], f32, tag="work", name=f"rsb_{gb}")
                    nc.tensor.matmul(rsb[:, :QS], lhsT=s_ind2, rhs=rsg,
                                     start=True, stop=True)
                    recipB = tmp.tile([128, QS], f32, tag="recip", name=f"recip_{gb}")
                    nc.vector.reciprocal_approx_fast(out=recipB, in_=rsb[:, :QS])
                    gt1 = tmp.tile([128, QS], f32, tag="gt1", name=f"gt1_{gb}")
                    nc.vector.tensor_tensor(gt1, acc[:, 0:QS], s_sigT[:, gb, :], MUL)
                    nc.vector.tensor_tensor(s_agT[:, gb, :], gt1, recipB, MUL)

            # ---- output projection ----
            s_outT = sb.tile([128, 2, QS], f32)
            for t in range(2):
                pt = pswork.tile([128, 512], f32, tag="work", name=f"o_ps_{t}")
                for gb in range(NG * 2):
                    nc.tensor.matmul(
                        pt[:, :QS], lhsT=s_ow[:, gb, t, :], rhs=s_agT[:, gb, :],
                        start=(gb == 0), stop=(gb == NG * 2 - 1),
                    )
                nc.scalar.add(s_outT[:, t, :], pt[:, :QS], s_ob[:, t:t + 1])
                nc.sync.dma_start(out=p_out[t], in_=s_outT[:, t, :])

    nc.finalize()
    return nc


_NC = None


def _get_nc():
    global _NC
    if _NC is None:
        _NC = build_kernel()
    return _NC


def kernel(**inputs) -> np.ndarray:
    nc = _get_nc()
    in_maps = make_in_maps(inputs)
    res = run_bass_kernel_spmd(nc, in_maps, core_ids=list(range(NCORES)))
    return gather_output(res.results)


def kernel_traced(**inputs):
    """Like kernel() but with NTFF profiling; returns (output, exec_time_ns, res)."""
    nc = _get_nc()
    in_maps = make_in_maps(inputs)
    res = run_bass_kernel_spmd(nc, in_maps, core_ids=list(range(NCORES)), trace=True)
    return gather_output(res.results), res.exec_time_ns, res

